# revision 104
# baseline (speedup 1.0000x reference)
"""AdaArcFace loss on 8 TRN2 NeuronCores (Bass, class-sharded tensor parallel).

loss = mean_i( LSE_i - 32*cosm_i ),  LSE_i = 32 + ln(S_i + em_i - ey_i)
  S_i  = sum_c exp(32*(cos[i,c] - 1))   <- the only term needing the big matmul
  cos_y/quantile/margin path is tiny, exact fp32, replicated on every core.

Sharding: 100000 classes -> 8 cores x 12544 (44 pad rows = -features[0], whose
softmax contribution is ~1e-17 relative). kernel shard is transposed on host
(layout only) so DMA streams contiguous and the PE gets emb-on-partitions.

v2: batch-on-partitions matmul layout. Stationary = f_hat^T chunks (8 total
weight loads per tile instead of 392), moving = wT f32r (1 cyc/row). Per-class
1/||w|| applied by DVE multiply against a gpsimd partition-broadcast row;
exp + per-sample partial sums come from one big ACT instruction per phase
(accum_out), eliminating the per-group S matmuls entirely.
"""

import math
import numpy as np

import concourse.bass as bass
import concourse.mybir as mybir
from concourse.bass_utils import run_bass_kernel_spmd

F32 = mybir.dt.float32
F32R = mybir.dt.float32r
BF16 = mybir.dt.bfloat16

# problem constants (hardcoded per harness contract)
B = 256          # batch
E = 512          # embedding
C = 100000       # classes
NCORES = 8
CPC = 12544      # classes per core (padded): 98 groups of 128
TILE_C = 1792    # classes per stream tile: 14 groups of 128
NTILES = CPC // TILE_C             # 7
GPT = TILE_C // 128                # 14 groups per tile
NCH = 4                            # 448-wide class chunks per tile
CHW = TILE_C // NCH                # 448
NPH = 2 * NTILES                   # 14 phases (bc-halves)
SCALE = 32.0
M_BASE = 0.5
ALPHA = 0.1
BETA = 0.15
SIN_M = math.sin(M_BASE)
LN32 = math.log(32.0)
ECH = E // 128   # 4 emb chunks

_CACHE = {}


def build_nc():
    nc = bass.Bass(target_bir_lowering=False, num_devices=NCORES)

    wt_ext = nc.declare_dram_parameter(
        "wt", [NTILES, ECH, 128, TILE_C], F32R, isOutput=False)
    feat_ext = nc.declare_dram_parameter("feat", [B, E], F32, isOutput=False)
    wlab_ext = nc.declare_dram_parameter("wlab", [B, E], F32, isOutput=False)
    out_ext = nc.declare_dram_parameter("out", [1, 1], F32, isOutput=True)

    cc_in = nc.dram_tensor("cc_in", [1, B], F32)
    cc_out = nc.dram_tensor("cc_out", [1, NCORES * B], F32, addr_space="Shared")
    invr_dram = nc.dram_tensor("invr_dram", [2, TILE_C], F32)

    WT_TILE_ELEMS = ECH * 128 * TILE_C

    from contextlib import ExitStack
    ctx = ExitStack()
    sb = lambda name, shape, dt=F32: ctx.enter_context(nc.sbuf_tensor(name, shape, dt))
    ps = lambda name, shape, dt=F32: ctx.enter_context(nc.psum_tensor(name, shape, dt))
    sem = lambda name: ctx.enter_context(nc.semaphore(name))

    with ctx:
        # --- SBUF ---
        WT = [sb(f"WT{i}", [128, ECH, TILE_C], F32R) for i in range(3)]
        W2 = sb("W2", [128, ECH, TILE_C], BF16)
        W2P = sb("W2P", [128, 2, TILE_C], BF16)
        W2S = [sb(f"W2S{i}", [128, TILE_C], BF16) for i in range(2)]
        LNQ = sb("LNQ", [128, GPT])
        INVC = [sb(f"INVC{i}", [128, GPT]) for i in range(2)]   # 32/||w|| columns
        INVB = [sb(f"INVB{i}", [128, TILE_C]) for i in range(2)]  # broadcast
        CN = [sb(f"CN{i}", [128, TILE_C]) for i in range(2)]    # 32*cos, per phase
        EJ = [sb(f"EJ{i}", [128, TILE_C], BF16) for i in range(2)]  # exp out (junk)
        SACC = sb("SACC", [128, 2, NTILES])                     # accum_out slots
        FT = sb("FT", [128, ECH, 2, 128], F32R)                 # fhatT: [e_p, ec, bc, b]
        F_ = sb("F", [128, 2, E])
        FN = sb("FN", [128, 2, E])
        WL = sb("WL", [128, 2, E])
        WLN = sb("WLN", [128, 2, E])
        CMP2 = sb("CMP2", [128, B])
        qf = sb("qf", [128, 2]); qw = sb("qw", [128, 2])
        rf = sb("rf", [128, 2]); rw = sb("rw", [128, 2])
        invf = sb("invf", [128, 2]); invw = sb("invw", [128, 2])
        cosy = sb("cosy", [128, 2]); dd = sb("dd", [128, 2])
        cnt = sb("cnt", [128, 2]); mask = sb("mask", [128, 2])
        t1 = sb("t1", [128, 2]); m015 = sb("m015", [128, 2]); mm_ = sb("mm", [128, 2])
        cmpv = sb("cmpv", [128, 2]); t2 = sb("t2", [128, 2]); t3 = sb("t3", [128, 2])
        cosm = sb("cosm", [128, 2]); ey = sb("ey", [128, 2]); em = sb("em", [128, 2])
        adj = sb("adj", [128, 2]); Sb = sb("Sb", [128, 2])
        drow = sb("drow", [1, B])
        TRS = sb("TRS", [1, 2 * B])     # [0:256]=cosm row, [256:512]=adj row
        li0 = sb("li0", [1, B])
        Ssb = sb("Ssb", [1, B])
        AGsb = sb("AGsb", [1, NCORES * B])
        Sfull = sb("Sfull", [1, B])
        TT = sb("TT", [1, B])
        lS = sb("lS", [1, B])
        li = sb("li", [1, B])
        lsum = sb("lsum", [1, 1])
        loss = sb("loss", [1, 1])
        ones128h = sb("ones128h", [128, 1], BF16)
        onesK1 = sb("onesK1", [1, 128])
        ONESCR = sb("ONESCR", [128, 128])
        ident = sb("ident", [128, 128])
        c_halfpi = sb("c_halfpi", [128, 1])  # -pi/2
        c_neg32 = sb("c_neg32", [128, 1])
        c_ln32 = sb("c_ln32", [128, 1])

        # --- PSUM: one 16KB tensor, manually laid out ---
        # D ping: chunks at 512*k (448 wide), k=0..3   [banks 0-3]
        # D pong: 2048 + 512*k                          [banks 4-7]
        # qcol ping: [1984:1998) (bank-3 slack); pong: [4032:4046) (bank-7 slack)
        # small path (before phase 0): PB@[0:256) TRXd@[512:768) TRXr@[1024:1536)
        # finale (after all phases):   TRS2@[512:768)
        PS = ps("PS", [128, 4096])
        Doff = lambda ph, k: (ph % 2) * 2048 + 512 * k
        QOFF = [1984, 4032]

        # --- semaphores ---
        s_inF = sem("s_inF"); s_inW = sem("s_inW"); s_gd = sem("s_gd")
        s_wtb = [sem(f"s_wtb{i}") for i in range(3)]
        s_cc = sem("s_cc"); s_const = sem("s_const"); s_gset = sem("s_gset")
        s_sq = sem("s_sq"); s_w2s = sem("s_w2s"); s_qmm = sem("s_qmm")
        s_lnq = sem("s_lnq"); s_gf = sem("s_gf"); s_invb = sem("s_invb")
        s_gp = sem("s_gp"); s_gfa = sem("s_gfa")
        s_D = sem("s_D"); s_mult = sem("s_mult"); s_exp = sem("s_exp")
        s_qfw = sem("s_qfw"); s_rec = sem("s_rec"); s_fn = sem("s_fn")
        s_inv = sem("s_inv"); s_vh = sem("s_vh"); s_ah = sem("s_ah")
        s_ftp = sem("s_ftp"); s_ftc = sem("s_ftc"); s_cy = sem("s_cy")
        s_dtr = sem("s_dtr"); s_drow = sem("s_drow"); s_db = sem("s_db")
        s_mask = sem("s_mask"); s_sin = sem("s_sin"); s_cosm = sem("s_cosm")
        s_eyem = sem("s_eyem"); s_adj = sem("s_adj"); s_tr2 = sem("s_tr2")
        s_rows = sem("s_rows"); s_sb = sem("s_sb"); s_str = sem("s_str")
        s_ssb = sem("s_ssb"); s_tt = sem("s_tt"); s_sfl = sem("s_sfl")
        s_lns = sem("s_lns"); s_loss = sem("s_loss")

        _hs = {"v": 0, "a": 0}

        def vbar(eng, ins):
            key = "v" if eng.engine == mybir.EngineType.DVE else "a"
            s = s_vh if key == "v" else s_ah
            _hs[key] += 1
            ins.then_inc(s, 1)
            eng.wait_ge(s, _hs[key])

        with nc.Block() as block:

            # ---------------- SYNC: input DMAs ----------------
            @block.sync
            def _(sync):
                sync.dma_start(
                    F_[:, :, :],
                    bass.AP(feat_ext, 0, [[E, 128], [128 * E, 2], [1, E]]),
                ).then_inc(s_inF, 16)
                sync.dma_start(
                    WL[:, :, :],
                    bass.AP(wlab_ext, 0, [[E, 128], [128 * E, 2], [1, E]]),
                ).then_inc(s_inW, 16)
                for t in range(NTILES):
                    if t >= 3:
                        sync.wait_ge(s_sq, t - 2)        # ACT squares of t-3 done
                        sync.wait_ge(s_D, 2 * (t - 2))   # PE phases of t-3 done
                    sync.dma_start(
                        WT[t % 3][:, :, :],
                        bass.AP(wt_ext, t * WT_TILE_ELEMS,
                                [[TILE_C, 128], [128 * TILE_C, ECH], [1, TILE_C]]),
                    ).then_inc(s_wtb[t % 3], 16)

            # ---------------- GPSIMD: consts, presum, inv bcast, collective ---
            @block.gpsimd
            def _(g):
                g.memset(ones128h[:, :], 1.0).then_inc(s_gset, 1)
                g.memset(onesK1[:, :], 1.0).then_inc(s_gset, 1)
                g.memset(ONESCR[:, :], 1.0).then_inc(s_gset, 1)
                g.memset(c_halfpi[:, :], -math.pi / 2.0).then_inc(s_gset, 1)
                g.memset(c_neg32[:, :], -SCALE).then_inc(s_gset, 1)
                g.memset(c_ln32[:, :], LN32).then_inc(s_gset, 1)
                g.wait_ge(s_gset, 6)
                g.affine_select(
                    ident[:, :], ONESCR[:, :], [[1, 128]],
                    compare_op=mybir.AluOpType.is_equal, fill=0.0,
                    base=0, channel_multiplier=-1,
                ).then_inc(s_const, 1)


                for t in range(NTILES):
                    # broadcast of the inv row (flatten DMA issued by ACT)
                    g.wait_ge(s_gfa, 16 * (t + 1))
                    if t >= 2:
                        g.wait_ge(s_mult, 2 * (t - 1))  # INVB buffer free
                    # broadcast row to all 128 partitions via 0-stride DMA read
                    g.dma_start(
                        INVB[t % 2][:, :],
                        bass.AP(invr_dram, (t % 2) * TILE_C,
                                [[0, 128], [1, TILE_C]]),
                    ).then_inc(s_gf, 16)
                    g.wait_ge(s_gf, 16 * (t + 1))
                    g.sem_inc(s_invb, 1)

                # collective + output
                g.wait_ge(s_ssb, 1)
                g.dma_start(cc_in[:, :], Ssb[:, :]).then_inc(s_gd, 16)
                g.wait_ge(s_gd, 16)
                g.collective_compute(
                    "AllGather", mybir.AluOpType.bypass,
                    replica_groups=[list(range(NCORES))],
                    ins=[cc_in.ap().opt()],
                    outs=[cc_out.ap().opt()],
                ).then_inc(s_cc, 1)
                g.wait_ge(s_cc, 1)
                g.dma_start(AGsb[:, :], cc_out[:, :]).then_inc(s_gd, 16)
                g.wait_ge(s_loss, 1)
                g.dma_start(out_ext[:, :], loss[:, :]).then_inc(s_gd, 16)
                g.wait_ge(s_gd, 48)

            # ---------------- ACT (scalar) ----------------
            @block.scalar
            def _(a):
                Act = mybir.ActivationFunctionType
                # small path: squared norms of f and wlab
                a.wait_ge(s_inF, 16)
                a.activation(CN[0][:, 0:E], F_[:, 0, :], Act.Square,
                             accum_out=qf[:, 0:1])
                a.activation(CN[0][:, E:2 * E], F_[:, 1, :], Act.Square,
                             accum_out=qf[:, 1:2])
                a.wait_ge(s_inW, 16)
                a.activation(CN[1][:, 0:E], WL[:, 0, :], Act.Square,
                             accum_out=qw[:, 0:1])
                a.activation(CN[1][:, E:2 * E], WL[:, 1, :], Act.Square,
                             accum_out=qw[:, 1:2]).then_inc(s_qfw, 1)
                a.wait_ge(s_rec, 1)
                a.activation(invf[:, :], rf[:, :], Act.Sqrt)
                a.activation(invw[:, :], rw[:, :], Act.Sqrt).then_inc(s_inv, 1)
                a.wait_ge(s_inv, 1)
                for b in range(2):
                    a.activation(FN[:, b, :], F_[:, b, :], Act.Copy,
                                 scale=invf[:, b:b + 1])
                for b in range(2):
                    ins = a.activation(WLN[:, b, :], WL[:, b, :], Act.Copy,
                                       scale=invw[:, b:b + 1])
                ins.then_inc(s_fn, 1)
                # margin path: cos(m*pi) = -sin(m*pi - pi/2), arg in [0, 1.1]
                a.wait_ge(s_mask, 1)
                a.activation(cmpv[:, :], mm_[:, :], Act.Sin,
                             bias=c_halfpi[:, :], scale=math.pi).then_inc(s_sin, 1)
                a.wait_ge(s_cosm, 1)
                a.activation(ey[:, :], cosy[:, :], Act.Exp,
                             bias=c_neg32[:, :], scale=SCALE)
                a.activation(em[:, :], cosm[:, :], Act.Exp,
                             bias=c_neg32[:, :], scale=SCALE).then_inc(s_eyem, 1)
                a.wait_ge(s_tr2, 1)
                a.activation(TRS[:, :], PS[0:1, 1024:1536],
                             Act.Copy).then_inc(s_rows, 1)

                # big loop: squares lead phases by 2 tiles; exps lag by 1
                def a_square(t):
                    a.wait_ge(s_wtb[t % 3], 16 * (t // 3 + 1))
                    if t >= 1:
                        a.wait_ge(s_gp, t)        # presum1(t-1) done reading W2
                    a.activation(W2[:, :, :], WT[t % 3][:, :, :].bitcast(F32),
                                 Act.Square).then_inc(s_sq, 1)

                def a_inv(t):
                    a.wait_ge(s_qmm, t + 1)
                    if t >= 1:
                        a.wait_ge(s_lnq, t)       # prior exp done reading LNQ
                    if t >= 2:
                        a.wait_ge(s_gfa, 16 * (t - 1))  # INVC buffer free
                    ins = a.activation(LNQ[:, :],
                                       PS[:, QOFF[t % 2]:QOFF[t % 2] + GPT],
                                       Act.Ln)
                    vbar(a, ins)
                    ins = a.activation(INVC[t % 2][:, :], LNQ[:, :], Act.Exp,
                                       bias=c_ln32[:, :], scale=-0.5)
                    ins.then_inc(s_lnq, 1)
                    a.wait_ge(s_lnq, t + 1)       # INVC visible before DMA read
                    if t >= 2:
                        a.wait_ge(s_invb, t - 1)  # invr_dram buffer free
                    # flatten inv columns (128,GPT)->(1,TILE_C) in class order
                    a.dma_start(
                        bass.AP(invr_dram, (t % 2) * TILE_C,
                                [[TILE_C, 1], [GPT, 128], [1, GPT]]),
                        bass.AP(INVC[t % 2], 0, [[GPT, 128], [GPT, 1], [1, GPT]]),
                    ).then_inc(s_gfa, 16)

                def a_exp(ph):
                    t, half = ph // 2, ph % 2
                    a.wait_ge(s_mult, ph + 1)
                    if ph >= 2:
                        a.wait_ge(s_exp, ph - 1)  # EJ buffer visible-order
                    a.activation(
                        EJ[ph % 2][:, :], CN[ph % 2][:, :], Act.Exp,
                        bias=c_neg32[:, :],
                        accum_out=bass.AP(
                            SACC, half * NTILES + t,
                            [[2 * NTILES, 128], [1, 1]])).then_inc(s_exp, 1)

                for t in range(NTILES):
                    a_square(t)
                    if t >= 1:
                        a_exp(2 * (t - 1))
                        a_exp(2 * (t - 1) + 1)
                    a_inv(t)
                a_exp(2 * (NTILES - 1))
                a_exp(2 * (NTILES - 1) + 1)

                # finale
                a.wait_ge(s_tt, 1)
                a.activation(lS[:, :], TT[:, :], Act.Ln).then_inc(s_lns, 1)

            # ---------------- DVE (vector) ----------------
            @block.vector
            def _(v):
                Alu = mybir.AluOpType
                v.wait_ge(s_qfw, 1)
                v.reciprocal(rf[:, :], qf[:, :])
                v.reciprocal(rw[:, :], qw[:, :]).then_inc(s_rec, 1)
                # fT chunk copies (ping-pong with PE transposes through PS[0:256))
                for ec in range(ECH):
                    v.wait_ge(s_ftp, ec + 1)
                    v.tensor_copy(
                        bass.AP(FT, ec * 256, [[ECH * 256, 128], [1, 256]]),
                        PS[:, 0:256]).then_inc(s_ftc, 1)
                # cos_y (exact fp32) and difficulty
                for b in range(2):
                    scrd = CN[b][:, 2 * E:3 * E]  # (128, 512) scratch
                    ins = v.tensor_mul(scrd, FN[:, b, :], WLN[:, b, :])
                    vbar(v, ins)
                    ins = v.tensor_reduce(cosy[:, b:b + 1], scrd,
                                          axis=mybir.AxisListType.X, op=Alu.add)
                    vbar(v, ins)
                v.tensor_scalar(dd[:, :], cosy[:, :], -1.0, 1.0,
                                Alu.mult, Alu.add).then_inc(s_cy, 1)
                v.wait_ge(s_dtr, 1)
                v.tensor_copy(drow[:, :], PS[0:1, 512:768]).then_inc(s_drow, 1)
                # rank/quantile: cnt_i = #{j: d_j <= d_i}; mask = cnt >= 52
                v.wait_ge(s_db, 1)
                for b in range(2):
                    ins = v.tensor_scalar(
                        CMP2[:, :], PS[:, 0:256], dd[:, b:b + 1], 0.0,
                        Alu.is_le, Alu.add, accum_out=cnt[:, b:b + 1])
                    vbar(v, ins)
                v.tensor_scalar(mask[:, :], cnt[:, :], 51.5, None, Alu.is_ge)
                ins = v.tensor_scalar(t1[:, :], dd[:, :], ALPHA, M_BASE,
                                      Alu.mult, Alu.add)
                vbar(v, ins)
                ins = v.tensor_scalar(m015[:, :], mask[:, :], BETA, None, Alu.mult)
                vbar(v, ins)
                v.tensor_add(mm_[:, :], t1[:, :], m015[:, :]).then_inc(s_mask, 1)
                v.wait_ge(s_sin, 1)
                v.tensor_mul(t2[:, :], cosy[:, :], cmpv[:, :])
                ins = v.tensor_scalar(t3[:, :], mm_[:, :], -SIN_M, None, Alu.mult)
                vbar(v, ins)
                v.tensor_sub(cosm[:, :], t3[:, :], t2[:, :]).then_inc(s_cosm, 1)
                v.wait_ge(s_eyem, 1)
                v.tensor_sub(adj[:, :], em[:, :], ey[:, :]).then_inc(s_adj, 1)
                v.wait_ge(s_rows, 1)
                v.tensor_scalar(li0[:, :], TRS[0:1, 0:B], -SCALE, SCALE,
                                Alu.mult, Alu.add)

                # big loop: presums (lead by 2) interleaved with phase multiplies
                def v_mult(ph):
                    t = ph // 2
                    v.wait_ge(s_D, ph + 1)
                    v.wait_ge(s_invb, t + 1)
                    if ph >= 2:
                        v.wait_ge(s_exp, ph - 1)   # CN buffer free
                    ins = None
                    for k in range(NCH):
                        ins = v.tensor_mul(
                            CN[ph % 2][:, k * CHW:(k + 1) * CHW],
                            PS[:, Doff(ph, k):Doff(ph, k) + CHW],
                            INVB[t % 2][:, k * CHW:(k + 1) * CHW])
                    ins.then_inc(s_mult, 1)

                for t in range(NTILES):
                    # presum the 4 emb-chunks of w^2 (bf16)
                    v.wait_ge(s_sq, t + 1)
                    if t >= 1:
                        v.wait_ge(s_w2s, t)       # presum2(t-1) done with W2P
                    v.tensor_add(W2P[:, :, :], W2[:, 0:2, :],
                                 W2[:, 2:4, :]).then_inc(s_gp, 1)
                    v.wait_ge(s_gp, t + 1)
                    if t >= 2:
                        v.wait_ge(s_qmm, t - 1)   # W2S buffer free
                    v.tensor_add(W2S[t % 2][:, :], W2P[:, 0, :],
                                 W2P[:, 1, :]).then_inc(s_w2s, 1)
                    v_mult(2 * t)
                    v_mult(2 * t + 1)

                # finale
                v.wait_ge(s_exp, NPH)
                ins = v.tensor_reduce(
                    Sb[:, :],
                    bass.AP(SACC, 0, [[2 * NTILES, 128], [NTILES, 2], [1, NTILES]]),
                    axis=mybir.AxisListType.X, op=Alu.add)
                ins.then_inc(s_sb, 1)
                v.wait_ge(s_str, 1)
                v.tensor_copy(Ssb[:, :], PS[0:1, 512:768]).then_inc(s_ssb, 1)
                v.wait_ge(s_gd, 32)
                ins = v.tensor_reduce(
                    Sfull[:, :],
                    bass.AP(AGsb, 0, [[NCORES * B, 1], [1, B], [B, NCORES]]),
                    axis=mybir.AxisListType.X, op=Alu.add)
                vbar(v, ins)
                v.tensor_add(TT[:, :], Sfull[:, :],
                             TRS[0:1, B:2 * B]).then_inc(s_tt, 1)
                v.wait_ge(s_lns, 1)
                ins = v.tensor_add(li[:, :], lS[:, :], li0[:, :])
                vbar(v, ins)
                ins = v.tensor_reduce(lsum[:, :], li[:, :],
                                      axis=mybir.AxisListType.X, op=Alu.add)
                vbar(v, ins)
                v.tensor_scalar(loss[:, :], lsum[:, :], 1.0 / B, None,
                                Alu.mult).then_inc(s_loss, 1)

            # ---------------- PE (tensor) ----------------
            @block.tensor
            def _(te):
                te.wait_ge(s_const, 1)
                te.wait_ge(s_fn, 1)
                # fT = transpose(f_norm): [e_p, ec, bc, b] via PS[0:256)
                for ec in range(ECH):
                    if ec >= 1:
                        te.wait_ge(s_ftc, ec)
                    te.transpose(PS[:, 0:128],
                                 FN[:, 0, ec * 128:(ec + 1) * 128], ident[:, :])
                    te.transpose(PS[:, 128:256],
                                 FN[:, 1, ec * 128:(ec + 1) * 128],
                                 ident[:, :]).then_inc(s_ftp, 1)
                # d column -> row (PS[512:768))
                te.wait_ge(s_cy, 1)
                te.transpose(PS[0:1, 512:640], dd[:, 0:1], ident[:, :])
                te.transpose(PS[0:1, 640:768], dd[:, 1:2],
                             ident[:, :]).then_inc(s_dtr, 1)
                # broadcast d row to 128 partitions (K=1 matmul into PS[0:256))
                te.wait_ge(s_drow, 1)
                te.wait_ge(s_ftc, ECH)
                te.matmul(PS[:, 0:256], onesK1[:, :], drow[:, :]).then_inc(s_db, 1)
                # cosm, adj columns -> rows (PS[1024:1536))
                te.wait_ge(s_adj, 1)
                te.transpose(PS[0:1, 1024:1152], cosm[:, 0:1], ident[:, :])
                te.transpose(PS[0:1, 1152:1280], cosm[:, 1:2], ident[:, :])
                te.transpose(PS[0:1, 1280:1408], adj[:, 0:1], ident[:, :])
                te.transpose(PS[0:1, 1408:1536], adj[:, 1:2],
                             ident[:, :]).then_inc(s_tr2, 1)

                # big loop
                def t_qmm(t):
                    # norm matmuls: q_g = ones^T @ W2S strided slice (bf16 FWL)
                    te.wait_ge(s_w2s, t + 1)
                    if t >= 2:
                        te.wait_ge(s_lnq, t - 1)   # qcol slack reuse
                    ins = None
                    for gi in range(GPT):
                        # strided class slice {GPT*p + gi}: column p of the
                        # norm output is class GPT*p+gi, so the (p,g) flatten
                        # lands in natural class order
                        ins = te.matmul(
                            PS[:, QOFF[t % 2] + gi:QOFF[t % 2] + gi + 1],
                            bass.AP(W2S[t % 2], gi, [[TILE_C, 128], [GPT, 128]]),
                            ones128h[:, :])
                    ins.then_inc(s_qmm, 1)

                def t_phase(ph):
                    t, half = ph // 2, ph % 2
                    if ph >= 2:
                        te.wait_ge(s_mult, ph - 1)  # D bank-set free
                    ins = None
                    for ec in range(ECH):
                        for k in range(NCH):
                            ins = te.matmul(
                                PS[:, Doff(ph, k):Doff(ph, k) + CHW],
                                FT[:, ec, half, :],
                                WT[t % 3][:, ec, k * CHW:(k + 1) * CHW],
                                start=(ec == 0), stop=(ec == ECH - 1),
                                skip_group_check=True)
                    ins.then_inc(s_D, 1)

                for t in range(NTILES):
                    t_qmm(t)
                    if t == 0:
                        # PS D-banks hold small-path data until these complete
                        te.wait_ge(s_mask, 1)
                        te.wait_ge(s_rows, 1)
                    t_phase(2 * t)
                    t_phase(2 * t + 1)

                # finale: Sb columns -> row (PS[512:768))
                te.wait_ge(s_sb, 1)
                te.transpose(PS[0:1, 512:640], Sb[:, 0:1], ident[:, :])
                te.transpose(PS[0:1, 640:768], Sb[:, 1:2],
                             ident[:, :]).then_inc(s_str, 1)


        return nc


def _shard_host(features, labels, kernel_w):
    """Host-side shard + pack (layout only, no arithmetic)."""
    features = np.ascontiguousarray(features, dtype=np.float32)
    kernel_w = np.ascontiguousarray(kernel_w, dtype=np.float32)
    labels = np.asarray(labels).astype(np.int64)
    wlab = np.ascontiguousarray(kernel_w[labels])        # (B, E) gather
    pad_row = -features[0]                               # direction only matters
    in_maps = []
    cpc_raw = C // NCORES                                # 12500
    for c in range(NCORES):
        shard = kernel_w[c * cpc_raw:(c + 1) * cpc_raw]  # (12500, E)
        pad = np.broadcast_to(pad_row, (CPC - cpc_raw, E))
        shard = np.concatenate([shard, pad], axis=0)     # (12544, E)
        # (CPC, E) -> transpose -> (E, CPC) -> (ECH,128, NTILES,TILE_C)
        wt = shard.T.reshape(ECH, 128, NTILES, TILE_C)
        wt = np.ascontiguousarray(wt.transpose(2, 0, 1, 3))  # (NTILES,ECH,128,TILE_C)
        in_maps.append({"wt": wt, "feat": features, "wlab": wlab})
    return in_maps


def _get_nc():
    if "nc" not in _CACHE:
        _CACHE["nc"] = build_nc()
    return _CACHE["nc"]


def kernel(features, labels, kernel):
    in_maps = _shard_host(features, labels, kernel)
    nc = _get_nc()
    res = run_bass_kernel_spmd(nc, in_maps, core_ids=list(range(NCORES)))
    out = res.results[0]["out"]
    return np.float32(out.reshape(())[()])


# revision 105
# speedup vs baseline: 1.2224x; 1.2224x over previous
"""AdaArcFace loss on 8 TRN2 NeuronCores (Bass, class-sharded tensor parallel).

loss = mean_i( LSE_i - 32*cosm_i ),  LSE_i = 32 + ln(S_i + em_i - ey_i)
  S_i  = sum_c exp(32*(cos[i,c] - 1))   <- the only term needing the big matmul
  cos_y/quantile/margin path is tiny, exact fp32, replicated on every core.

Sharding: 100000 classes -> 8 cores x 12544 (44 pad rows = -features[0], whose
softmax contribution is ~1e-17 relative). kernel shard is transposed on host
(layout only) so DMA streams contiguous and the PE gets emb-on-partitions.

v2: batch-on-partitions matmul layout. Stationary = f_hat^T chunks (8 total
weight loads per tile instead of 392), moving = wT f32r (1 cyc/row). Per-class
1/||w|| applied by DVE multiply against a gpsimd partition-broadcast row;
exp + per-sample partial sums come from one big ACT instruction per phase
(accum_out), eliminating the per-group S matmuls entirely.
"""

import math
import numpy as np

import concourse.bass as bass
import concourse.mybir as mybir
from concourse.bass_utils import run_bass_kernel_spmd

F32 = mybir.dt.float32
F32R = mybir.dt.float32r
BF16 = mybir.dt.bfloat16

# problem constants (hardcoded per harness contract)
B = 256          # batch
E = 512          # embedding
C = 100000       # classes
NCORES = 8
CPC = 12544      # classes per core (padded): 98 groups of 128
TILE_C = 1792    # classes per stream tile: 14 groups of 128
NTILES = CPC // TILE_C             # 7
GPT = TILE_C // 128                # 14 groups per tile
NCH = 4                            # 448-wide class chunks per tile
CHW = TILE_C // NCH                # 448
NPH = 2 * NTILES                   # 14 phases (bc-halves)
SCALE = 32.0
M_BASE = 0.5
ALPHA = 0.1
BETA = 0.15
SIN_M = math.sin(M_BASE)
LN32 = math.log(32.0)
ECH = E // 128   # 4 emb chunks

_CACHE = {}


def build_nc():
    nc = bass.Bass(target_bir_lowering=False, num_devices=NCORES)

    wt_ext = nc.declare_dram_parameter(
        "wt", [NTILES, ECH, 128, TILE_C], F32R, isOutput=False)
    feat_ext = nc.declare_dram_parameter("feat", [B, E], F32, isOutput=False)
    wlab_ext = nc.declare_dram_parameter("wlab", [B, E], F32, isOutput=False)
    out_ext = nc.declare_dram_parameter("out", [1, 1], F32, isOutput=True)

    cc_in = nc.dram_tensor("cc_in", [1, B], F32)
    cc_out = nc.dram_tensor("cc_out", [1, NCORES * B], F32, addr_space="Shared")
    invr_dram = nc.dram_tensor("invr_dram", [2, TILE_C], F32)

    WT_TILE_ELEMS = ECH * 128 * TILE_C

    from contextlib import ExitStack
    ctx = ExitStack()
    sb = lambda name, shape, dt=F32: ctx.enter_context(nc.sbuf_tensor(name, shape, dt))
    ps = lambda name, shape, dt=F32: ctx.enter_context(nc.psum_tensor(name, shape, dt))
    sem = lambda name: ctx.enter_context(nc.semaphore(name))

    with ctx:
        # --- SBUF ---
        WT = [sb(f"WT{i}", [128, ECH, TILE_C], F32R) for i in range(3)]
        W2 = sb("W2", [128, ECH, TILE_C], BF16)
        W2P = sb("W2P", [128, 2, TILE_C], BF16)
        W2S = [sb(f"W2S{i}", [128, TILE_C], BF16) for i in range(2)]
        LNQ = sb("LNQ", [128, GPT])
        INVC = [sb(f"INVC{i}", [128, GPT]) for i in range(2)]   # 32/||w|| columns
        INVB = [sb(f"INVB{i}", [128, TILE_C]) for i in range(2)]  # broadcast
        CN = [sb(f"CN{i}", [128, TILE_C]) for i in range(2)]    # 32*cos, per phase
        EJ = [sb(f"EJ{i}", [128, TILE_C], BF16) for i in range(2)]  # exp out (junk)
        SACC = sb("SACC", [128, 2, NTILES])                     # accum_out slots
        FT = sb("FT", [128, ECH, 2, 128], F32R)                 # fhatT: [e_p, ec, bc, b]
        F_ = sb("F", [128, 2, E])
        FN = sb("FN", [128, 2, E])
        WL = sb("WL", [128, 2, E])
        WLN = sb("WLN", [128, 2, E])
        CMP2 = sb("CMP2", [128, B])
        qf = sb("qf", [128, 2]); qw = sb("qw", [128, 2])
        rf = sb("rf", [128, 2]); rw = sb("rw", [128, 2])
        invf = sb("invf", [128, 2]); invw = sb("invw", [128, 2])
        cosy = sb("cosy", [128, 2]); dd = sb("dd", [128, 2])
        cnt = sb("cnt", [128, 2]); mask = sb("mask", [128, 2])
        t1 = sb("t1", [128, 2]); m015 = sb("m015", [128, 2]); mm_ = sb("mm", [128, 2])
        cmpv = sb("cmpv", [128, 2]); t2 = sb("t2", [128, 2]); t3 = sb("t3", [128, 2])
        cosm = sb("cosm", [128, 2]); ey = sb("ey", [128, 2]); em = sb("em", [128, 2])
        adj = sb("adj", [128, 2]); Sb = sb("Sb", [128, 2])
        drow = sb("drow", [1, B])
        TRS = sb("TRS", [1, 2 * B])     # [0:256]=cosm row, [256:512]=adj row
        li0 = sb("li0", [1, B])
        Ssb = sb("Ssb", [1, B])
        AGsb = sb("AGsb", [1, NCORES * B])
        Sfull = sb("Sfull", [1, B])
        TT = sb("TT", [1, B])
        lS = sb("lS", [1, B])
        li = sb("li", [1, B])
        lsum = sb("lsum", [1, 1])
        loss = sb("loss", [1, 1])
        ones128h = sb("ones128h", [128, 1], BF16)
        onesK1 = sb("onesK1", [1, 128])
        ONESCR = sb("ONESCR", [128, 128])
        ident = sb("ident", [128, 128])
        c_halfpi = sb("c_halfpi", [128, 1])  # -pi/2
        c_neg32 = sb("c_neg32", [128, 1])
        c_ln32 = sb("c_ln32", [128, 1])

        # --- PSUM: one 16KB tensor, manually laid out ---
        # D ping: chunks at 512*k (448 wide), k=0..3   [banks 0-3]
        # D pong: 2048 + 512*k                          [banks 4-7]
        # qcol ping: [1984:1998) (bank-3 slack); pong: [4032:4046) (bank-7 slack)
        # small path (before phase 0): PB@[0:256) TRXd@[512:768) TRXr@[1024:1536)
        # finale (after all phases):   TRS2@[512:768)
        PS = ps("PS", [128, 4096])
        Doff = lambda ph, k: (ph % 2) * 2048 + 512 * k
        QOFF = [1984, 4032]

        # --- semaphores ---
        s_inF = sem("s_inF"); s_inW = sem("s_inW"); s_gd = sem("s_gd")
        s_wtb = [sem(f"s_wtb{i}") for i in range(3)]
        s_cc = sem("s_cc"); s_const = sem("s_const"); s_gset = sem("s_gset")
        s_sq = sem("s_sq"); s_w2s = sem("s_w2s"); s_qmm = sem("s_qmm")
        s_lnq = sem("s_lnq"); s_gf = sem("s_gf"); s_invb = sem("s_invb")
        s_gp = sem("s_gp")
        s_D = sem("s_D"); s_mult = sem("s_mult"); s_exp = sem("s_exp")
        s_qfw = sem("s_qfw"); s_rec = sem("s_rec"); s_fn = sem("s_fn")
        s_inv = sem("s_inv"); s_vh = sem("s_vh"); s_ah = sem("s_ah")
        s_ftp = sem("s_ftp"); s_ftc = sem("s_ftc"); s_cy = sem("s_cy")
        s_dtr = sem("s_dtr"); s_drow = sem("s_drow"); s_db = sem("s_db")
        s_mask = sem("s_mask"); s_sin = sem("s_sin"); s_cosm = sem("s_cosm")
        s_eyem = sem("s_eyem"); s_adj = sem("s_adj"); s_tr2 = sem("s_tr2")
        s_rows = sem("s_rows"); s_sb = sem("s_sb"); s_str = sem("s_str")
        s_ssb = sem("s_ssb"); s_tt = sem("s_tt"); s_sfl = sem("s_sfl")
        s_lns = sem("s_lns"); s_loss = sem("s_loss")

        _hs = {"v": 0, "a": 0}

        def vbar(eng, ins):
            key = "v" if eng.engine == mybir.EngineType.DVE else "a"
            s = s_vh if key == "v" else s_ah
            _hs[key] += 1
            ins.then_inc(s, 1)
            eng.wait_ge(s, _hs[key])

        with nc.Block() as block:

            # ---------------- SYNC: input DMAs ----------------
            @block.sync
            def _(sync):
                sync.dma_start(
                    F_[:, :, :],
                    bass.AP(feat_ext, 0, [[E, 128], [128 * E, 2], [1, E]]),
                ).then_inc(s_inF, 16)
                sync.dma_start(
                    WL[:, :, :],
                    bass.AP(wlab_ext, 0, [[E, 128], [128 * E, 2], [1, E]]),
                ).then_inc(s_inW, 16)
                for t in range(NTILES):
                    if t >= 3:
                        sync.wait_ge(s_sq, t - 2)        # ACT squares of t-3 done
                        sync.wait_ge(s_D, 2 * (t - 2))   # PE phases of t-3 done
                    sync.dma_start(
                        WT[t % 3][:, :, :],
                        bass.AP(wt_ext, t * WT_TILE_ELEMS,
                                [[TILE_C, 128], [128 * TILE_C, ECH], [1, TILE_C]]),
                    ).then_inc(s_wtb[t % 3], 16)

            # ---------------- GPSIMD: consts, presum, inv bcast, collective ---
            @block.gpsimd
            def _(g):
                g.memset(ones128h[:, :], 1.0).then_inc(s_gset, 1)
                g.memset(onesK1[:, :], 1.0).then_inc(s_gset, 1)
                g.memset(ONESCR[:, :], 1.0).then_inc(s_gset, 1)
                g.memset(c_halfpi[:, :], -math.pi / 2.0).then_inc(s_gset, 1)
                g.memset(c_neg32[:, :], -SCALE).then_inc(s_gset, 1)
                g.memset(c_ln32[:, :], LN32).then_inc(s_gset, 1)
                g.wait_ge(s_gset, 6)
                g.affine_select(
                    ident[:, :], ONESCR[:, :], [[1, 128]],
                    compare_op=mybir.AluOpType.is_equal, fill=0.0,
                    base=0, channel_multiplier=-1,
                ).then_inc(s_const, 1)


                for t in range(NTILES):
                    # inv row: flatten columns (128,GPT)->(1,TILE_C), broadcast
                    g.wait_ge(s_lnq, t + 1)
                    if t >= 2:
                        g.wait_ge(s_invb, t - 1)  # INVR buffer free
                    # class j <-> (p=j//GPT, g=j%GPT): contiguous on both sides
                    g.dma_start(
                        bass.AP(invr_dram, (t % 2) * TILE_C,
                                [[TILE_C, 1], [GPT, 128], [1, GPT]]),
                        bass.AP(INVC[t % 2], 0, [[GPT, 128], [GPT, 1], [1, GPT]]),
                    ).then_inc(s_gf, 16)
                    g.wait_ge(s_gf, 32 * t + 16)
                    if t >= 2:
                        g.wait_ge(s_mult, 2 * (t - 1))  # INVB buffer free
                    # broadcast row to all 128 partitions via 0-stride DMA read
                    g.dma_start(
                        INVB[t % 2][:, :],
                        bass.AP(invr_dram, (t % 2) * TILE_C,
                                [[0, 128], [1, TILE_C]]),
                    ).then_inc(s_gf, 16)
                    g.wait_ge(s_gf, 32 * (t + 1))
                    g.sem_inc(s_invb, 1)

                # collective + output
                g.wait_ge(s_ssb, 1)
                g.dma_start(cc_in[:, :], Ssb[:, :]).then_inc(s_gd, 16)
                g.wait_ge(s_gd, 16)
                g.collective_compute(
                    "AllGather", mybir.AluOpType.bypass,
                    replica_groups=[list(range(NCORES))],
                    ins=[cc_in.ap().opt()],
                    outs=[cc_out.ap().opt()],
                ).then_inc(s_cc, 1)
                g.wait_ge(s_cc, 1)
                g.dma_start(AGsb[:, :], cc_out[:, :]).then_inc(s_gd, 16)
                g.wait_ge(s_loss, 1)
                g.dma_start(out_ext[:, :], loss[:, :]).then_inc(s_gd, 16)
                g.wait_ge(s_gd, 48)

            # ---------------- ACT (scalar) ----------------
            @block.scalar
            def _(a):
                Act = mybir.ActivationFunctionType
                # small path: squared norms of f and wlab
                a.wait_ge(s_inF, 16)
                a.activation(CN[0][:, 0:E], F_[:, 0, :], Act.Square,
                             accum_out=qf[:, 0:1])
                a.activation(CN[0][:, E:2 * E], F_[:, 1, :], Act.Square,
                             accum_out=qf[:, 1:2])
                a.wait_ge(s_inW, 16)
                a.activation(CN[1][:, 0:E], WL[:, 0, :], Act.Square,
                             accum_out=qw[:, 0:1])
                a.activation(CN[1][:, E:2 * E], WL[:, 1, :], Act.Square,
                             accum_out=qw[:, 1:2]).then_inc(s_qfw, 1)
                a.wait_ge(s_rec, 1)
                a.activation(invf[:, :], rf[:, :], Act.Sqrt)
                a.activation(invw[:, :], rw[:, :], Act.Sqrt).then_inc(s_inv, 1)
                a.wait_ge(s_inv, 1)
                for b in range(2):
                    a.activation(FN[:, b, :], F_[:, b, :], Act.Copy,
                                 scale=invf[:, b:b + 1])
                for b in range(2):
                    ins = a.activation(WLN[:, b, :], WL[:, b, :], Act.Copy,
                                       scale=invw[:, b:b + 1])
                ins.then_inc(s_fn, 1)
                # margin path: cos(m*pi) = -sin(m*pi - pi/2), arg in [0, 1.1]
                a.wait_ge(s_mask, 1)
                a.activation(cmpv[:, :], mm_[:, :], Act.Sin,
                             bias=c_halfpi[:, :], scale=math.pi).then_inc(s_sin, 1)
                a.wait_ge(s_cosm, 1)
                a.activation(ey[:, :], cosy[:, :], Act.Exp,
                             bias=c_neg32[:, :], scale=SCALE)
                a.activation(em[:, :], cosm[:, :], Act.Exp,
                             bias=c_neg32[:, :], scale=SCALE).then_inc(s_eyem, 1)
                a.wait_ge(s_tr2, 1)
                a.activation(TRS[:, :], PS[0:1, 1024:1536],
                             Act.Copy).then_inc(s_rows, 1)

                # big loop: squares lead phases by 2 tiles; exps lag by 1
                def a_square(t):
                    a.wait_ge(s_wtb[t % 3], 16 * (t // 3 + 1))
                    if t >= 1:
                        a.wait_ge(s_gp, t)        # presum1(t-1) done reading W2
                    a.activation(W2[:, :, :], WT[t % 3][:, :, :].bitcast(F32),
                                 Act.Square).then_inc(s_sq, 1)

                def a_inv(t):
                    a.wait_ge(s_qmm, t + 1)
                    if t >= 1:
                        a.wait_ge(s_lnq, t)       # prior exp done reading LNQ
                    if t >= 2:
                        a.wait_ge(s_gf, 32 * (t - 2) + 16)  # INVC buffer free
                    ins = a.activation(LNQ[:, :],
                                       PS[:, QOFF[t % 2]:QOFF[t % 2] + GPT],
                                       Act.Ln)
                    vbar(a, ins)
                    a.activation(INVC[t % 2][:, :], LNQ[:, :], Act.Exp,
                                 bias=c_ln32[:, :], scale=-0.5).then_inc(s_lnq, 1)

                def a_exp(ph):
                    t, half = ph // 2, ph % 2
                    a.wait_ge(s_mult, ph + 1)
                    if ph >= 2:
                        a.wait_ge(s_exp, ph - 1)  # EJ buffer visible-order
                    a.activation(
                        EJ[ph % 2][:, :], CN[ph % 2][:, :], Act.Exp,
                        bias=c_neg32[:, :],
                        accum_out=bass.AP(
                            SACC, half * NTILES + t,
                            [[2 * NTILES, 128], [1, 1]])).then_inc(s_exp, 1)

                for t in range(NTILES):
                    a_square(t)
                    if t >= 1:
                        a_exp(2 * (t - 1))
                        a_exp(2 * (t - 1) + 1)
                    a_inv(t)
                a_exp(2 * (NTILES - 1))
                a_exp(2 * (NTILES - 1) + 1)

                # finale
                a.wait_ge(s_tt, 1)
                a.activation(lS[:, :], TT[:, :], Act.Ln).then_inc(s_lns, 1)

            # ---------------- DVE (vector) ----------------
            @block.vector
            def _(v):
                Alu = mybir.AluOpType
                v.wait_ge(s_qfw, 1)
                v.reciprocal(rf[:, :], qf[:, :])
                v.reciprocal(rw[:, :], qw[:, :]).then_inc(s_rec, 1)
                # fT chunk copies (ping-pong with PE transposes through PS[0:256))
                for ec in range(ECH):
                    v.wait_ge(s_ftp, ec + 1)
                    v.tensor_copy(
                        bass.AP(FT, ec * 256, [[ECH * 256, 128], [1, 256]]),
                        PS[:, 0:256]).then_inc(s_ftc, 1)
                # cos_y (exact fp32) and difficulty
                for b in range(2):
                    scrd = CN[b][:, 2 * E:3 * E]  # (128, 512) scratch
                    ins = v.tensor_mul(scrd, FN[:, b, :], WLN[:, b, :])
                    vbar(v, ins)
                    ins = v.tensor_reduce(cosy[:, b:b + 1], scrd,
                                          axis=mybir.AxisListType.X, op=Alu.add)
                    vbar(v, ins)
                v.tensor_scalar(dd[:, :], cosy[:, :], -1.0, 1.0,
                                Alu.mult, Alu.add).then_inc(s_cy, 1)
                v.wait_ge(s_dtr, 1)
                v.tensor_copy(drow[:, :], PS[0:1, 512:768]).then_inc(s_drow, 1)
                # rank/quantile: cnt_i = #{j: d_j <= d_i}; mask = cnt >= 52
                v.wait_ge(s_db, 1)
                for b in range(2):
                    ins = v.tensor_scalar(
                        CMP2[:, :], PS[:, 0:256], dd[:, b:b + 1], 0.0,
                        Alu.is_le, Alu.add, accum_out=cnt[:, b:b + 1])
                    vbar(v, ins)
                v.tensor_scalar(mask[:, :], cnt[:, :], 51.5, None, Alu.is_ge)
                ins = v.tensor_scalar(t1[:, :], dd[:, :], ALPHA, M_BASE,
                                      Alu.mult, Alu.add)
                vbar(v, ins)
                ins = v.tensor_scalar(m015[:, :], mask[:, :], BETA, None, Alu.mult)
                vbar(v, ins)
                v.tensor_add(mm_[:, :], t1[:, :], m015[:, :]).then_inc(s_mask, 1)
                v.wait_ge(s_sin, 1)
                v.tensor_mul(t2[:, :], cosy[:, :], cmpv[:, :])
                ins = v.tensor_scalar(t3[:, :], mm_[:, :], -SIN_M, None, Alu.mult)
                vbar(v, ins)
                v.tensor_sub(cosm[:, :], t3[:, :], t2[:, :]).then_inc(s_cosm, 1)
                v.wait_ge(s_eyem, 1)
                v.tensor_sub(adj[:, :], em[:, :], ey[:, :]).then_inc(s_adj, 1)
                v.wait_ge(s_rows, 1)
                v.tensor_scalar(li0[:, :], TRS[0:1, 0:B], -SCALE, SCALE,
                                Alu.mult, Alu.add)

                # big loop: presums (lead by 2) interleaved with phase multiplies
                def v_mult(ph):
                    t = ph // 2
                    v.wait_ge(s_D, ph + 1)
                    v.wait_ge(s_invb, t + 1)
                    if ph >= 2:
                        v.wait_ge(s_exp, ph - 1)   # CN buffer free
                    ins = None
                    for k in range(NCH):
                        ins = v.tensor_mul(
                            CN[ph % 2][:, k * CHW:(k + 1) * CHW],
                            PS[:, Doff(ph, k):Doff(ph, k) + CHW],
                            INVB[t % 2][:, k * CHW:(k + 1) * CHW])
                    ins.then_inc(s_mult, 1)

                for t in range(NTILES):
                    # presum the 4 emb-chunks of w^2 (bf16)
                    v.wait_ge(s_sq, t + 1)
                    if t >= 1:
                        v.wait_ge(s_w2s, t)       # presum2(t-1) done with W2P
                    v.tensor_add(W2P[:, :, :], W2[:, 0:2, :],
                                 W2[:, 2:4, :]).then_inc(s_gp, 1)
                    v.wait_ge(s_gp, t + 1)
                    if t >= 2:
                        v.wait_ge(s_qmm, t - 1)   # W2S buffer free
                    v.tensor_add(W2S[t % 2][:, :], W2P[:, 0, :],
                                 W2P[:, 1, :]).then_inc(s_w2s, 1)
                    v_mult(2 * t)
                    v_mult(2 * t + 1)

                # finale
                v.wait_ge(s_exp, NPH)
                ins = v.tensor_reduce(
                    Sb[:, :],
                    bass.AP(SACC, 0, [[2 * NTILES, 128], [NTILES, 2], [1, NTILES]]),
                    axis=mybir.AxisListType.X, op=Alu.add)
                ins.then_inc(s_sb, 1)
                v.wait_ge(s_str, 1)
                v.tensor_copy(Ssb[:, :], PS[0:1, 512:768]).then_inc(s_ssb, 1)
                v.wait_ge(s_gd, 32)
                ins = v.tensor_reduce(
                    Sfull[:, :],
                    bass.AP(AGsb, 0, [[NCORES * B, 1], [1, B], [B, NCORES]]),
                    axis=mybir.AxisListType.X, op=Alu.add)
                vbar(v, ins)
                v.tensor_add(TT[:, :], Sfull[:, :],
                             TRS[0:1, B:2 * B]).then_inc(s_tt, 1)
                v.wait_ge(s_lns, 1)
                ins = v.tensor_add(li[:, :], lS[:, :], li0[:, :])
                vbar(v, ins)
                ins = v.tensor_reduce(lsum[:, :], li[:, :],
                                      axis=mybir.AxisListType.X, op=Alu.add)
                vbar(v, ins)
                v.tensor_scalar(loss[:, :], lsum[:, :], 1.0 / B, None,
                                Alu.mult).then_inc(s_loss, 1)

            # ---------------- PE (tensor) ----------------
            @block.tensor
            def _(te):
                te.wait_ge(s_const, 1)
                te.wait_ge(s_fn, 1)
                # fT = transpose(f_norm): [e_p, ec, bc, b] via PS[0:256)
                for ec in range(ECH):
                    if ec >= 1:
                        te.wait_ge(s_ftc, ec)
                    te.transpose(PS[:, 0:128],
                                 FN[:, 0, ec * 128:(ec + 1) * 128], ident[:, :])
                    te.transpose(PS[:, 128:256],
                                 FN[:, 1, ec * 128:(ec + 1) * 128],
                                 ident[:, :]).then_inc(s_ftp, 1)
                # d column -> row (PS[512:768))
                te.wait_ge(s_cy, 1)
                te.transpose(PS[0:1, 512:640], dd[:, 0:1], ident[:, :])
                te.transpose(PS[0:1, 640:768], dd[:, 1:2],
                             ident[:, :]).then_inc(s_dtr, 1)
                # broadcast d row to 128 partitions (K=1 matmul into PS[0:256))
                te.wait_ge(s_drow, 1)
                te.wait_ge(s_ftc, ECH)
                te.matmul(PS[:, 0:256], onesK1[:, :], drow[:, :]).then_inc(s_db, 1)
                # cosm, adj columns -> rows (PS[1024:1536))
                te.wait_ge(s_adj, 1)
                te.transpose(PS[0:1, 1024:1152], cosm[:, 0:1], ident[:, :])
                te.transpose(PS[0:1, 1152:1280], cosm[:, 1:2], ident[:, :])
                te.transpose(PS[0:1, 1280:1408], adj[:, 0:1], ident[:, :])
                te.transpose(PS[0:1, 1408:1536], adj[:, 1:2],
                             ident[:, :]).then_inc(s_tr2, 1)

                # big loop
                def t_qmm(t):
                    # norm matmuls: q_g = ones^T @ W2S strided slice (bf16 FWL)
                    te.wait_ge(s_w2s, t + 1)
                    if t >= 2:
                        te.wait_ge(s_lnq, t - 1)   # qcol slack reuse
                    ins = None
                    for gi in range(GPT):
                        # strided class slice {GPT*p + gi}: column p of the
                        # norm output is class GPT*p+gi, so the (p,g) flatten
                        # lands in natural class order
                        ins = te.matmul(
                            PS[:, QOFF[t % 2] + gi:QOFF[t % 2] + gi + 1],
                            bass.AP(W2S[t % 2], gi, [[TILE_C, 128], [GPT, 128]]),
                            ones128h[:, :])
                    ins.then_inc(s_qmm, 1)

                def t_phase(ph):
                    t, half = ph // 2, ph % 2
                    if ph >= 2:
                        te.wait_ge(s_mult, ph - 1)  # D bank-set free
                    ins = None
                    for ec in range(ECH):
                        for k in range(NCH):
                            ins = te.matmul(
                                PS[:, Doff(ph, k):Doff(ph, k) + CHW],
                                FT[:, ec, half, :],
                                WT[t % 3][:, ec, k * CHW:(k + 1) * CHW],
                                start=(ec == 0), stop=(ec == ECH - 1),
                                skip_group_check=True)
                    ins.then_inc(s_D, 1)

                te.wait_ge(s_mask, 1)   # PS[0:256) free (quantile compares done)
                te.wait_ge(s_rows, 1)   # PS[1024:1536) copied out
                for t in range(NTILES):
                    t_qmm(t)
                    t_phase(2 * t)
                    t_phase(2 * t + 1)

                # finale: Sb columns -> row (PS[512:768))
                te.wait_ge(s_sb, 1)
                te.transpose(PS[0:1, 512:640], Sb[:, 0:1], ident[:, :])
                te.transpose(PS[0:1, 640:768], Sb[:, 1:2],
                             ident[:, :]).then_inc(s_str, 1)


        return nc


def _shard_host(features, labels, kernel_w):
    """Host-side shard + pack (layout only, no arithmetic)."""
    features = np.ascontiguousarray(features, dtype=np.float32)
    kernel_w = np.ascontiguousarray(kernel_w, dtype=np.float32)
    labels = np.asarray(labels).astype(np.int64)
    wlab = np.ascontiguousarray(kernel_w[labels])        # (B, E) gather
    pad_row = -features[0]                               # direction only matters
    in_maps = []
    cpc_raw = C // NCORES                                # 12500
    for c in range(NCORES):
        shard = kernel_w[c * cpc_raw:(c + 1) * cpc_raw]  # (12500, E)
        pad = np.broadcast_to(pad_row, (CPC - cpc_raw, E))
        shard = np.concatenate([shard, pad], axis=0)     # (12544, E)
        # (CPC, E) -> transpose -> (E, CPC) -> (ECH,128, NTILES,TILE_C)
        wt = shard.T.reshape(ECH, 128, NTILES, TILE_C)
        wt = np.ascontiguousarray(wt.transpose(2, 0, 1, 3))  # (NTILES,ECH,128,TILE_C)
        in_maps.append({"wt": wt, "feat": features, "wlab": wlab})
    return in_maps


def _get_nc():
    if "nc" not in _CACHE:
        _CACHE["nc"] = build_nc()
    return _CACHE["nc"]


def kernel(features, labels, kernel):
    in_maps = _shard_host(features, labels, kernel)
    nc = _get_nc()
    res = run_bass_kernel_spmd(nc, in_maps, core_ids=list(range(NCORES)))
    out = res.results[0]["out"]
    return np.float32(out.reshape(())[()])


# revision 108
# speedup vs baseline: 1.2321x; 1.0080x over previous
"""AdaArcFace loss on 8 TRN2 NeuronCores (Bass, class-sharded tensor parallel).

loss = mean_i( LSE_i - 32*cosm_i ),  LSE_i = 32 + ln(S_i + em_i - ey_i)
  S_i  = sum_c exp(32*(cos[i,c] - 1))   <- the only term needing the big matmul
  cos_y/quantile/margin path is tiny, exact fp32, replicated on every core.

Sharding: 100000 classes -> 8 cores x 12544 (44 pad rows = -features[0], whose
softmax contribution is ~1e-17 relative). kernel shard is transposed on host
(layout only) so DMA streams contiguous and the PE gets emb-on-partitions.

v2: batch-on-partitions matmul layout. Stationary = f_hat^T chunks (8 total
weight loads per tile instead of 392), moving = wT f32r (1 cyc/row). Per-class
1/||w|| applied by DVE multiply against a gpsimd partition-broadcast row;
exp + per-sample partial sums come from one big ACT instruction per phase
(accum_out), eliminating the per-group S matmuls entirely.
"""

import math
import numpy as np

import concourse.bass as bass
import concourse.mybir as mybir
from concourse.bass_utils import run_bass_kernel_spmd

F32 = mybir.dt.float32
F32R = mybir.dt.float32r
BF16 = mybir.dt.bfloat16

# problem constants (hardcoded per harness contract)
B = 256          # batch
E = 512          # embedding
C = 100000       # classes
NCORES = 8
CPC = 12544      # classes per core (padded): 98 groups of 128
TILE_C = 1792    # classes per stream tile: 14 groups of 128
NTILES = CPC // TILE_C             # 7
GPT = TILE_C // 128                # 14 groups per tile
NCH = 4                            # 448-wide class chunks per tile
CHW = TILE_C // NCH                # 448
NPH = 2 * NTILES                   # 14 phases (bc-halves)
SCALE = 32.0
M_BASE = 0.5
ALPHA = 0.1
BETA = 0.15
SIN_M = math.sin(M_BASE)
LN32 = math.log(32.0)
ECH = E // 128   # 4 emb chunks

_CACHE = {}


def build_nc():
    nc = bass.Bass(target_bir_lowering=False, num_devices=NCORES)

    wt_ext = nc.declare_dram_parameter(
        "wt", [NTILES, ECH, 128, TILE_C], F32R, isOutput=False)
    feat_ext = nc.declare_dram_parameter("feat", [B, E], F32, isOutput=False)
    wlab_ext = nc.declare_dram_parameter("wlab", [B, E], F32, isOutput=False)
    out_ext = nc.declare_dram_parameter("out", [1, 1], F32, isOutput=True)

    cc_in = nc.dram_tensor("cc_in", [1, B], F32)
    cc_out = nc.dram_tensor("cc_out", [1, NCORES * B], F32, addr_space="Shared")
    invr_dram = nc.dram_tensor("invr_dram", [2, TILE_C], F32)

    WT_TILE_ELEMS = ECH * 128 * TILE_C

    from contextlib import ExitStack
    ctx = ExitStack()
    sb = lambda name, shape, dt=F32: ctx.enter_context(nc.sbuf_tensor(name, shape, dt))
    ps = lambda name, shape, dt=F32: ctx.enter_context(nc.psum_tensor(name, shape, dt))
    sem = lambda name: ctx.enter_context(nc.semaphore(name))

    with ctx:
        # --- SBUF ---
        WT = [sb(f"WT{i}", [128, ECH, TILE_C], F32R) for i in range(3)]
        W2 = sb("W2", [128, ECH, TILE_C], BF16)
        W2P = sb("W2P", [128, 2, TILE_C], BF16)
        W2S = [sb(f"W2S{i}", [128, TILE_C], BF16) for i in range(2)]
        LNQ = sb("LNQ", [128, GPT])
        INVC = [sb(f"INVC{i}", [128, GPT]) for i in range(2)]   # 32/||w|| columns
        INVB = [sb(f"INVB{i}", [128, TILE_C]) for i in range(2)]  # broadcast
        CN = [sb(f"CN{i}", [128, TILE_C]) for i in range(2)]    # 32*cos, per phase
        EJ = [sb(f"EJ{i}", [128, TILE_C], BF16) for i in range(2)]  # exp out (junk)
        SACC = sb("SACC", [128, 2, NTILES])                     # accum_out slots
        FT = sb("FT", [128, ECH, 2, 128], F32R)                 # fhatT: [e_p, ec, bc, b]
        F_ = sb("F", [128, 2, E])
        FN = sb("FN", [128, 2, E])
        WL = sb("WL", [128, 2, E])
        WLN = sb("WLN", [128, 2, E])
        CMP2 = sb("CMP2", [128, B])
        qf = sb("qf", [128, 2]); qw = sb("qw", [128, 2])
        rf = sb("rf", [128, 2]); rw = sb("rw", [128, 2])
        invf = sb("invf", [128, 2]); invw = sb("invw", [128, 2])
        cosy = sb("cosy", [128, 2]); dd = sb("dd", [128, 2])
        cnt = sb("cnt", [128, 2]); mask = sb("mask", [128, 2])
        t1 = sb("t1", [128, 2]); m015 = sb("m015", [128, 2]); mm_ = sb("mm", [128, 2])
        cmpv = sb("cmpv", [128, 2]); t2 = sb("t2", [128, 2]); t3 = sb("t3", [128, 2])
        cosm = sb("cosm", [128, 2]); ey = sb("ey", [128, 2]); em = sb("em", [128, 2])
        adj = sb("adj", [128, 2]); Sb = sb("Sb", [128, 2])
        drow = sb("drow", [1, B])
        TRS = sb("TRS", [1, 2 * B])     # [0:256]=cosm row, [256:512]=adj row
        li0 = sb("li0", [1, B])
        Ssb = sb("Ssb", [1, B])
        AGsb = sb("AGsb", [1, NCORES * B])
        Sfull = sb("Sfull", [1, B])
        TT = sb("TT", [1, B])
        lS = sb("lS", [1, B])
        li = sb("li", [1, B])
        lsum = sb("lsum", [1, 1])
        loss = sb("loss", [1, 1])
        ones128h = sb("ones128h", [128, 1], BF16)
        onesK1 = sb("onesK1", [1, 128])
        ONESCR = sb("ONESCR", [128, 128])
        ident = sb("ident", [128, 128])
        c_halfpi = sb("c_halfpi", [128, 1])  # -pi/2
        c_neg32 = sb("c_neg32", [128, 1])
        c_ln32 = sb("c_ln32", [128, 1])

        # --- PSUM: one 16KB tensor, manually laid out ---
        # D ping: chunks at 512*k (448 wide), k=0..3   [banks 0-3]
        # D pong: 2048 + 512*k                          [banks 4-7]
        # qcol ping: [1984:1998) (bank-3 slack); pong: [4032:4046) (bank-7 slack)
        # small path (before phase 0): PB@[0:256) TRXd@[512:768) TRXr@[1024:1536)
        # finale (after all phases):   TRS2@[512:768)
        PS = ps("PS", [128, 4096])
        Doff = lambda ph, k: (ph % 2) * 2048 + 512 * k
        QOFF = [1984, 4032]

        # --- semaphores ---
        s_inF = sem("s_inF"); s_inW = sem("s_inW"); s_gd = sem("s_gd")
        s_wtb = [sem(f"s_wtb{i}") for i in range(3)]
        s_cc = sem("s_cc"); s_const = sem("s_const"); s_gset = sem("s_gset")
        s_sq = sem("s_sq"); s_w2s = sem("s_w2s"); s_qmm = sem("s_qmm")
        s_lnq = sem("s_lnq"); s_gf = sem("s_gf"); s_invb = sem("s_invb")
        s_gp = sem("s_gp")
        s_D = sem("s_D"); s_mult = sem("s_mult"); s_exp = sem("s_exp")
        s_qfw = sem("s_qfw"); s_rec = sem("s_rec"); s_fn = sem("s_fn")
        s_inv = sem("s_inv"); s_vh = sem("s_vh"); s_ah = sem("s_ah")
        s_ftp = sem("s_ftp"); s_ftc = sem("s_ftc"); s_cy = sem("s_cy")
        s_dtr = sem("s_dtr"); s_drow = sem("s_drow"); s_db = sem("s_db")
        s_mask = sem("s_mask"); s_sin = sem("s_sin"); s_cosm = sem("s_cosm")
        s_eyem = sem("s_eyem"); s_adj = sem("s_adj"); s_tr2 = sem("s_tr2")
        s_rows = sem("s_rows"); s_sb = sem("s_sb"); s_str = sem("s_str")
        s_ssb = sem("s_ssb"); s_tt = sem("s_tt"); s_sfl = sem("s_sfl")
        s_lns = sem("s_lns"); s_loss = sem("s_loss")

        _hs = {"v": 0, "a": 0}

        def vbar(eng, ins):
            key = "v" if eng.engine == mybir.EngineType.DVE else "a"
            s = s_vh if key == "v" else s_ah
            _hs[key] += 1
            ins.then_inc(s, 1)
            eng.wait_ge(s, _hs[key])

        with nc.Block() as block:

            # ---------------- SYNC: input DMAs ----------------
            @block.sync
            def _(sync):
                sync.dma_start(
                    F_[:, :, :],
                    bass.AP(feat_ext, 0, [[E, 128], [128 * E, 2], [1, E]]),
                ).then_inc(s_inF, 16)
                sync.dma_start(
                    WL[:, :, :],
                    bass.AP(wlab_ext, 0, [[E, 128], [128 * E, 2], [1, E]]),
                ).then_inc(s_inW, 16)
                for t in range(NTILES):
                    if t >= 3:
                        sync.wait_ge(s_sq, t - 2)        # ACT squares of t-3 done
                        sync.wait_ge(s_D, 2 * (t - 2))   # PE phases of t-3 done
                    sync.dma_start(
                        WT[t % 3][:, :, :],
                        bass.AP(wt_ext, t * WT_TILE_ELEMS,
                                [[TILE_C, 128], [128 * TILE_C, ECH], [1, TILE_C]]),
                    ).then_inc(s_wtb[t % 3], 16)

            # ---------------- GPSIMD: consts, presum, inv bcast, collective ---
            @block.gpsimd
            def _(g):
                g.memset(ones128h[:, :], 1.0).then_inc(s_gset, 1)
                g.memset(onesK1[:, :], 1.0).then_inc(s_gset, 1)
                g.memset(ONESCR[:, :], 1.0).then_inc(s_gset, 1)
                g.memset(c_halfpi[:, :], -math.pi / 2.0).then_inc(s_gset, 1)
                g.memset(c_neg32[:, :], -SCALE).then_inc(s_gset, 1)
                g.memset(c_ln32[:, :], LN32).then_inc(s_gset, 1)
                g.wait_ge(s_gset, 6)
                g.affine_select(
                    ident[:, :], ONESCR[:, :], [[1, 128]],
                    compare_op=mybir.AluOpType.is_equal, fill=0.0,
                    base=0, channel_multiplier=-1,
                ).then_inc(s_const, 1)


                for t in range(NTILES):
                    # inv row: flatten columns (128,GPT)->(1,TILE_C), broadcast
                    g.wait_ge(s_lnq, t + 1)
                    if t >= 2:
                        g.wait_ge(s_invb, t - 1)  # INVR buffer free
                    # class j <-> (p=j//GPT, g=j%GPT): contiguous on both sides
                    g.dma_start(
                        bass.AP(invr_dram, (t % 2) * TILE_C,
                                [[TILE_C, 1], [GPT, 128], [1, GPT]]),
                        bass.AP(INVC[t % 2], 0, [[GPT, 128], [GPT, 1], [1, GPT]]),
                    ).then_inc(s_gf, 16)
                    g.wait_ge(s_gf, 32 * t + 16)
                    if t >= 2:
                        g.wait_ge(s_mult, 2 * (t - 1))  # INVB buffer free
                    # broadcast row to all 128 partitions via 0-stride DMA read
                    g.dma_start(
                        INVB[t % 2][:, :],
                        bass.AP(invr_dram, (t % 2) * TILE_C,
                                [[0, 128], [1, TILE_C]]),
                    ).then_inc(s_gf, 16)
                    g.wait_ge(s_gf, 32 * (t + 1))
                    g.sem_inc(s_invb, 1)

                # collective + output
                g.wait_ge(s_ssb, 1)
                g.dma_start(cc_in[:, :], Ssb[:, :]).then_inc(s_gd, 16)
                g.wait_ge(s_gd, 16)
                g.collective_compute(
                    "AllGather", mybir.AluOpType.bypass,
                    replica_groups=[list(range(NCORES))],
                    ins=[cc_in.ap().opt()],
                    outs=[cc_out.ap().opt()],
                ).then_inc(s_cc, 1)
                g.wait_ge(s_cc, 1)
                g.dma_start(AGsb[:, :], cc_out[:, :]).then_inc(s_gd, 16)
                g.wait_ge(s_loss, 1)
                g.dma_start(out_ext[:, :], loss[:, :]).then_inc(s_gd, 16)
                g.wait_ge(s_gd, 48)

            # ---------------- ACT (scalar) ----------------
            @block.scalar
            def _(a):
                Act = mybir.ActivationFunctionType
                # small path: squared norms of f and wlab
                a.wait_ge(s_inF, 16)
                a.activation(CN[0][:, 0:E], F_[:, 0, :], Act.Square,
                             accum_out=qf[:, 0:1])
                a.activation(CN[0][:, E:2 * E], F_[:, 1, :], Act.Square,
                             accum_out=qf[:, 1:2])
                a.wait_ge(s_inW, 16)
                a.activation(CN[1][:, 0:E], WL[:, 0, :], Act.Square,
                             accum_out=qw[:, 0:1])
                a.activation(CN[1][:, E:2 * E], WL[:, 1, :], Act.Square,
                             accum_out=qw[:, 1:2]).then_inc(s_qfw, 1)
                a.wait_ge(s_rec, 1)
                a.activation(invf[:, :], rf[:, :], Act.Sqrt)
                a.activation(invw[:, :], rw[:, :], Act.Sqrt).then_inc(s_inv, 1)
                a.wait_ge(s_inv, 1)
                for b in range(2):
                    a.activation(FN[:, b, :], F_[:, b, :], Act.Copy,
                                 scale=invf[:, b:b + 1])
                for b in range(2):
                    ins = a.activation(WLN[:, b, :], WL[:, b, :], Act.Copy,
                                       scale=invw[:, b:b + 1])
                ins.then_inc(s_fn, 1)
                # margin path: cos(m*pi) = -sin(m*pi - pi/2), arg in [0, 1.1]
                a.wait_ge(s_mask, 1)
                a.activation(cmpv[:, :], mm_[:, :], Act.Sin,
                             bias=c_halfpi[:, :], scale=math.pi).then_inc(s_sin, 1)
                a.wait_ge(s_cosm, 1)
                a.activation(ey[:, :], cosy[:, :], Act.Exp,
                             bias=c_neg32[:, :], scale=SCALE)
                a.activation(em[:, :], cosm[:, :], Act.Exp,
                             bias=c_neg32[:, :], scale=SCALE).then_inc(s_eyem, 1)
                a.wait_ge(s_tr2, 1)
                a.activation(TRS[:, :], PS[0:1, 1024:1536],
                             Act.Copy).then_inc(s_rows, 1)

                # big loop: squares lead phases by 2 tiles; exps lag by 1
                def a_square(t):
                    a.wait_ge(s_wtb[t % 3], 16 * (t // 3 + 1))
                    if t >= 1:
                        a.wait_ge(s_gp, t)        # presum1(t-1) done reading W2
                    a.activation(W2[:, :, :], WT[t % 3][:, :, :].bitcast(F32),
                                 Act.Square).then_inc(s_sq, 1)

                def a_inv(t):
                    a.wait_ge(s_qmm, t + 1)
                    if t >= 1:
                        a.wait_ge(s_lnq, t)       # prior exp done reading LNQ
                    if t >= 2:
                        a.wait_ge(s_gf, 32 * (t - 2) + 16)  # INVC buffer free
                    ins = a.activation(LNQ[:, :],
                                       PS[:, QOFF[t % 2]:QOFF[t % 2] + GPT],
                                       Act.Ln)
                    vbar(a, ins)
                    a.activation(INVC[t % 2][:, :], LNQ[:, :], Act.Exp,
                                 bias=c_ln32[:, :], scale=-0.5).then_inc(s_lnq, 1)

                def a_exp(ph):
                    t, half = ph // 2, ph % 2
                    a.wait_ge(s_mult, ph + 1)
                    if ph >= 2:
                        a.wait_ge(s_exp, ph - 1)  # EJ buffer visible-order
                    a.activation(
                        EJ[ph % 2][:, :], CN[ph % 2][:, :], Act.Exp,
                        bias=c_neg32[:, :],
                        accum_out=bass.AP(
                            SACC, half * NTILES + t,
                            [[2 * NTILES, 128], [1, 1]])).then_inc(s_exp, 1)

                for t in range(NTILES):
                    a_square(t)
                    if t >= 1:
                        a_exp(2 * (t - 1))
                        a_exp(2 * (t - 1) + 1)
                    a_inv(t)
                a_exp(2 * (NTILES - 1))
                a_exp(2 * (NTILES - 1) + 1)

                # finale
                a.wait_ge(s_tt, 1)
                a.activation(lS[:, :], TT[:, :], Act.Ln).then_inc(s_lns, 1)

            # ---------------- DVE (vector) ----------------
            @block.vector
            def _(v):
                Alu = mybir.AluOpType
                v.wait_ge(s_qfw, 1)
                v.reciprocal(rf[:, :], qf[:, :])
                v.reciprocal(rw[:, :], qw[:, :]).then_inc(s_rec, 1)
                # fT chunk copies (ping-pong with PE transposes through PS[0:256))
                for ec in range(ECH):
                    v.wait_ge(s_ftp, ec + 1)
                    v.tensor_copy(
                        bass.AP(FT, ec * 256, [[ECH * 256, 128], [1, 256]]),
                        PS[:, 0:256]).then_inc(s_ftc, 1)
                # cos_y (exact fp32) and difficulty
                for b in range(2):
                    scrd = CN[b][:, 2 * E:3 * E]  # (128, 512) scratch
                    ins = v.tensor_mul(scrd, FN[:, b, :], WLN[:, b, :])
                    vbar(v, ins)
                    ins = v.tensor_reduce(cosy[:, b:b + 1], scrd,
                                          axis=mybir.AxisListType.X, op=Alu.add)
                    vbar(v, ins)
                v.tensor_scalar(dd[:, :], cosy[:, :], -1.0, 1.0,
                                Alu.mult, Alu.add).then_inc(s_cy, 1)
                v.wait_ge(s_dtr, 1)
                v.tensor_copy(drow[:, :], PS[0:1, 512:768]).then_inc(s_drow, 1)
                # rank/quantile: cnt_i = #{j: d_j <= d_i}; mask = cnt >= 52
                v.wait_ge(s_db, 1)
                for b in range(2):
                    ins = v.tensor_scalar(
                        CMP2[:, :], PS[:, 0:256], dd[:, b:b + 1], 0.0,
                        Alu.is_le, Alu.add, accum_out=cnt[:, b:b + 1])
                    vbar(v, ins)
                v.tensor_scalar(mask[:, :], cnt[:, :], 51.5, None, Alu.is_ge)
                ins = v.tensor_scalar(t1[:, :], dd[:, :], ALPHA, M_BASE,
                                      Alu.mult, Alu.add)
                vbar(v, ins)
                ins = v.tensor_scalar(m015[:, :], mask[:, :], BETA, None, Alu.mult)
                vbar(v, ins)
                v.tensor_add(mm_[:, :], t1[:, :], m015[:, :]).then_inc(s_mask, 1)
                v.wait_ge(s_sin, 1)
                v.tensor_mul(t2[:, :], cosy[:, :], cmpv[:, :])
                ins = v.tensor_scalar(t3[:, :], mm_[:, :], -SIN_M, None, Alu.mult)
                vbar(v, ins)
                v.tensor_sub(cosm[:, :], t3[:, :], t2[:, :]).then_inc(s_cosm, 1)
                v.wait_ge(s_eyem, 1)
                v.tensor_sub(adj[:, :], em[:, :], ey[:, :]).then_inc(s_adj, 1)
                v.wait_ge(s_rows, 1)
                v.tensor_scalar(li0[:, :], TRS[0:1, 0:B], -SCALE, SCALE,
                                Alu.mult, Alu.add)

                # big loop: presums (lead by 2) interleaved with phase multiplies
                def v_mult(ph):
                    t = ph // 2
                    v.wait_ge(s_D, ph + 1)
                    v.wait_ge(s_invb, t + 1)
                    if ph >= 2:
                        v.wait_ge(s_exp, ph - 1)   # CN buffer free
                    ins = None
                    for k in range(NCH):
                        ins = v.tensor_mul(
                            CN[ph % 2][:, k * CHW:(k + 1) * CHW],
                            PS[:, Doff(ph, k):Doff(ph, k) + CHW],
                            INVB[t % 2][:, k * CHW:(k + 1) * CHW])
                    ins.then_inc(s_mult, 1)

                for t in range(NTILES):
                    # presum the 4 emb-chunks of w^2 (bf16)
                    v.wait_ge(s_sq, t + 1)
                    if t >= 1:
                        v.wait_ge(s_w2s, t)       # presum2(t-1) done with W2P
                    v.tensor_add(W2P[:, :, :], W2[:, 0:2, :],
                                 W2[:, 2:4, :]).then_inc(s_gp, 1)
                    v.wait_ge(s_gp, t + 1)
                    if t >= 2:
                        v.wait_ge(s_qmm, t - 1)   # W2S buffer free
                    v.tensor_add(W2S[t % 2][:, :], W2P[:, 0, :],
                                 W2P[:, 1, :]).then_inc(s_w2s, 1)
                    v_mult(2 * t)
                    v_mult(2 * t + 1)

                # finale
                v.wait_ge(s_exp, NPH)
                ins = v.tensor_reduce(
                    Sb[:, :],
                    bass.AP(SACC, 0, [[2 * NTILES, 128], [NTILES, 2], [1, NTILES]]),
                    axis=mybir.AxisListType.X, op=Alu.add)
                ins.then_inc(s_sb, 1)
                v.wait_ge(s_str, 1)
                v.tensor_copy(Ssb[:, :], PS[0:1, 512:768]).then_inc(s_ssb, 1)
                v.wait_ge(s_gd, 32)
                ins = v.tensor_reduce(
                    Sfull[:, :],
                    bass.AP(AGsb, 0, [[NCORES * B, 1], [1, B], [B, NCORES]]),
                    axis=mybir.AxisListType.X, op=Alu.add)
                vbar(v, ins)
                v.tensor_add(TT[:, :], Sfull[:, :],
                             TRS[0:1, B:2 * B]).then_inc(s_tt, 1)
                v.wait_ge(s_lns, 1)
                ins = v.tensor_add(li[:, :], lS[:, :], li0[:, :])
                vbar(v, ins)
                ins = v.tensor_reduce(lsum[:, :], li[:, :],
                                      axis=mybir.AxisListType.X, op=Alu.add)
                vbar(v, ins)
                v.tensor_scalar(loss[:, :], lsum[:, :], 1.0 / B, None,
                                Alu.mult).then_inc(s_loss, 1)

            # ---------------- PE (tensor) ----------------
            @block.tensor
            def _(te):
                te.wait_ge(s_const, 1)
                te.wait_ge(s_fn, 1)
                # fT = transpose(f_norm): [e_p, ec, bc, b] via PS[0:256)
                for ec in range(ECH):
                    if ec >= 1:
                        te.wait_ge(s_ftc, ec)
                    te.transpose(PS[:, 0:128],
                                 FN[:, 0, ec * 128:(ec + 1) * 128], ident[:, :])
                    te.transpose(PS[:, 128:256],
                                 FN[:, 1, ec * 128:(ec + 1) * 128],
                                 ident[:, :]).then_inc(s_ftp, 1)
                # d column -> row (PS[512:768))
                te.wait_ge(s_cy, 1)
                te.transpose(PS[0:1, 512:640], dd[:, 0:1], ident[:, :])
                te.transpose(PS[0:1, 640:768], dd[:, 1:2],
                             ident[:, :]).then_inc(s_dtr, 1)
                # broadcast d row to 128 partitions (K=1 matmul into PS[0:256))
                te.wait_ge(s_drow, 1)
                te.wait_ge(s_ftc, ECH)
                te.matmul(PS[:, 0:256], onesK1[:, :], drow[:, :]).then_inc(s_db, 1)
                # cosm, adj columns -> rows (PS[1024:1536))
                te.wait_ge(s_adj, 1)
                te.transpose(PS[0:1, 1024:1152], cosm[:, 0:1], ident[:, :])
                te.transpose(PS[0:1, 1152:1280], cosm[:, 1:2], ident[:, :])
                te.transpose(PS[0:1, 1280:1408], adj[:, 0:1], ident[:, :])
                te.transpose(PS[0:1, 1408:1536], adj[:, 1:2],
                             ident[:, :]).then_inc(s_tr2, 1)

                # big loop
                def t_qmm(t):
                    # norm matmuls: q_g = ones^T @ W2S strided slice (bf16 FWL)
                    te.wait_ge(s_w2s, t + 1)
                    if t >= 2:
                        te.wait_ge(s_lnq, t - 1)   # qcol slack reuse
                    ins = None
                    for gi in range(GPT):
                        # strided class slice {GPT*p + gi}: column p of the
                        # norm output is class GPT*p+gi, so the (p,g) flatten
                        # lands in natural class order
                        ins = te.matmul(
                            PS[:, QOFF[t % 2] + gi:QOFF[t % 2] + gi + 1],
                            bass.AP(W2S[t % 2], gi, [[TILE_C, 128], [GPT, 128]]),
                            ones128h[:, :])
                    ins.then_inc(s_qmm, 1)

                def t_phase(ph):
                    t, half = ph // 2, ph % 2
                    if ph >= 2:
                        te.wait_ge(s_mult, ph - 1)  # D bank-set free
                    ins = None
                    for ec in range(ECH):
                        for k in range(NCH):
                            ins = te.matmul(
                                PS[:, Doff(ph, k):Doff(ph, k) + CHW],
                                FT[:, ec, half, :],
                                WT[t % 3][:, ec, k * CHW:(k + 1) * CHW],
                                start=(ec == 0), stop=(ec == ECH - 1),
                                skip_group_check=True)
                    ins.then_inc(s_D, 1)

                te.wait_ge(s_mask, 1)   # PS[0:256) free (quantile compares done)
                te.wait_ge(s_rows, 1)   # PS[1024:1536) copied out
                for t in range(NTILES):
                    t_qmm(t)
                    t_phase(2 * t)
                    t_phase(2 * t + 1)

                # finale: Sb columns -> row (PS[512:768))
                te.wait_ge(s_sb, 1)
                te.transpose(PS[0:1, 512:640], Sb[:, 0:1], ident[:, :])
                te.transpose(PS[0:1, 640:768], Sb[:, 1:2],
                             ident[:, :]).then_inc(s_str, 1)


        return nc


def _shard_host(features, labels, kernel_w):
    """Host-side shard + pack (layout only, no arithmetic)."""
    features = np.ascontiguousarray(features, dtype=np.float32)
    kernel_w = np.ascontiguousarray(kernel_w, dtype=np.float32)
    labels = np.asarray(labels).astype(np.int64)
    wlab = np.ascontiguousarray(kernel_w[labels])        # (B, E) gather
    pad_row = -features[0]                               # direction only matters
    in_maps = []
    cpc_raw = C // NCORES                                # 12500
    for c in range(NCORES):
        shard = kernel_w[c * cpc_raw:(c + 1) * cpc_raw]  # (12500, E)
        pad = np.broadcast_to(pad_row, (CPC - cpc_raw, E))
        shard = np.concatenate([shard, pad], axis=0)     # (12544, E)
        # (CPC, E) -> transpose -> (E, CPC) -> (ECH,128, NTILES,TILE_C)
        wt = shard.T.reshape(ECH, 128, NTILES, TILE_C)
        wt = np.ascontiguousarray(wt.transpose(2, 0, 1, 3))  # (NTILES,ECH,128,TILE_C)
        in_maps.append({"wt": wt, "feat": features, "wlab": wlab})
    return in_maps


def _get_nc():
    if "nc" not in _CACHE:
        _CACHE["nc"] = build_nc()
    return _CACHE["nc"]


def kernel(features, labels, kernel):
    in_maps = _shard_host(features, labels, kernel)
    nc = _get_nc()
    res = run_bass_kernel_spmd(nc, in_maps, core_ids=list(range(NCORES)))
    out = res.results[0]["out"]
    return np.float32(out.reshape(())[()])
